# revision 26
# baseline (speedup 1.0000x reference)
"""AttentionPool Trainium2 kernel (8-core SPMD, batch-sharded, fp8-e3m4).

Math (algebraically folded from the reference):
  The single learned query collapses attention to:
    ws[h,:]   = sum_{d in head h} q_flat[h*64+d] * wk[h*64+d, :]   (host, tiny)
    s[b,h,n]  = tokens[b,n,:] @ (ws[h,:] * scale)                  (device)
    p         = exp(s)            (softmax shift cancels; |s| <~ 3, safe)
    pooled    = (p @ tokens) / sum_n p                             (device)
    ctx[b,e]  = wv[e,:] @ pooled[b, e//64, :];  out = ctx @ out_w.T + c
  Per-head score bias is constant within each softmax row and cancels; all
  other biases fold into c = out_w @ bv + out_b (host).

Device design (per core: 4 batches x 4096 tokens):
  - Tokens streamed twice in fp8 e3m4 (4-bit mantissa): natural layout
    [ki, m, d] for the pooled matmul and host-pre-transposed [ki, j, n] for
    the scores matmul.  25 MB/core total, 1.5 MB per DMA, 12 KB contiguous
    per partition line.
  - Scores for each 2048-token tile land stacked in ONE psum bank:
    chunk c (512 tokens) occupies partitions 32c..32c+16 (12 real heads + 4
    zero-pad).  A single accumulation group spans all 24 matmuls of a tile
    (per-element has_written handles the disjoint partition regions), so exp
    is one [128,512] ACT op and p-transposes are 4x [128,128] per tile.
  - ws is scaled x128 on host before e3m4 (its entries otherwise sit in the
    subnormal range); exp descales via its free `scale` parameter.  p is
    scaled x0.5 via the exp `bias` so its e3m4 cast can't overflow (max 15.5);
    the softmax normalization cancels the factor.
  - Tail: wv/out_w projections in bf16, normalization folded past the wv
    projection via a 0/1 head-selection matmul on 1/l.
"""

import numpy as np

P = 128
D = 768
H = 12
HP = 16          # heads padded to 16 (psum partition block)
DJ = D // P      # 6 chunks of the model dim
C = 2048         # tokens per DMA tile
CH = 512         # tokens per score chunk (one psum bank column span)
NCH = C // CH    # 4 chunks per tile
MB = C // P      # 16 token blocks of 128 per tile
B = 32
N = 4096
NT = N // C      # tiles per batch
NCORES = 8
BLOC = B // NCORES
WS_SCALE = 1.0
P_SCALE = 1.0
WARMUP = 0

_PATCHED = False


def _patch_tile_drain():
    """This walrus build allows only ONE sync wait per instruction (2 for
    EventSemaphore), but TileContext._drain_and_barrier puts a wait per
    outstanding semaphore on the single tail Drain. Split: one Drain each."""
    global _PATCHED
    if _PATCHED:
        return
    import bass_rust
    import concourse.tile as tile
    from concourse.vector_clock import ScopedClock

    def _drain_and_barrier(self, tick_clock, wait_clock):
        nc = self.nc
        probe = nc.sync.drain()
        wait_clock.add_sem_waits(
            probe.ins, ScopedClock({None: tick_clock.global_clock})
        )
        si = probe.ins.sync_info
        if si is not None and len(si.on_wait) > 1:
            waits = list(si.on_wait)
            probe.ins.sync_info = bass_rust.SyncInfo(
                on_wait=[waits[0]], on_update=list(si.on_update)
            )
            for w in waits[1:]:
                extra = nc.sync.drain()
                extra.ins.sync_info = bass_rust.SyncInfo(on_wait=[w], on_update=[])
        nc.all_engine_barrier()
        popped = nc._tile_sem_poison_stack.pop()
        assert popped is self._sem_poison
        nc.clear_and_free_semaphores(list(self.sems.allocated().values()))
        nc.all_engine_barrier()

    tile.TileContext._drain_and_barrier = _drain_and_barrier
    _PATCHED = True


def _legalize_waits(nc):
    """TRN2 walrus encodes at most ONE sync wait per instruction (two for
    EventSemaphore). Tile's wait assignment can leave more; hoist the extras
    onto standalone EventSemaphore instructions inserted just before, on the
    same engine (same semantics: engine blocks on them in order)."""
    import bass_rust
    from concourse import mybir

    n_fixed = 0
    for f in nc.m.functions:
        for bb in f.blocks:
            out = []
            for inst in bb.instructions:
                si = inst.sync_info
                waits = list(si.on_wait) if si is not None else []
                cap = 2 if isinstance(inst, mybir.InstEventSemaphore) else 1
                if len(waits) > cap:
                    extras, keep = waits[:-cap], waits[-cap:]
                    for i in range(0, len(extras), 2):
                        ev = mybir.InstEventSemaphore(
                            name=f"EVW-{inst.name}-{i}", ins=[], outs=[]
                        )
                        ev.engine = inst.engine
                        ev.sync_info = bass_rust.SyncInfo(
                            on_wait=extras[i : i + 2], on_update=[]
                        )
                        out.append(ev)
                    inst.sync_info = bass_rust.SyncInfo(
                        on_wait=keep, on_update=list(si.on_update)
                    )
                    n_fixed += 1
                out.append(inst)
            bb.instructions = out
    return n_fixed


def build_nc(bloc=BLOC, nt=NT, ws_bf16=False, legalize=True):
    import concourse.bass as bass
    import concourse.tile as tile
    from concourse import mybir
    from concourse.masks import make_identity

    f32 = mybir.dt.float32
    f32r = mybir.dt.float32r
    f16 = mybir.dt.float16
    bf16 = mybir.dt.bfloat16
    f8 = mybir.dt.float8e3
    EXP = mybir.ActivationFunctionType.Exp
    NTILES = bloc * nt

    nc = bass.Bass()
    tokn = nc.declare_dram_parameter("tokn", [NTILES, P, MB * D], f8, isOutput=False)
    tokt = nc.declare_dram_parameter("tokt", [NTILES, P, DJ * C], f8, isOutput=False)
    wsdt = bf16 if ws_bf16 else f16
    wsd = nc.declare_dram_parameter("wsd", [P, DJ * 32], wsdt, isOutput=False)
    wvT = nc.declare_dram_parameter("wvT", [P, DJ * D], f16, isOutput=False)
    owT = nc.declare_dram_parameter("owT", [P, DJ * D], f16, isOutput=False)
    cvec = nc.declare_dram_parameter("cvec", [bloc, D], f32, isOutput=False)
    out_d = nc.declare_dram_parameter("out", [bloc, D], f32, isOutput=True)

    with tile.TileContext(nc) as tc:
        with (
            tc.tile_pool(name="singles", bufs=1) as singles,
            tc.tile_pool(name="tokn", bufs=4) as tokn_pool,
            tc.tile_pool(name="tokt", bufs=4) as tokt_pool,
            tc.tile_pool(name="pp", bufs=3) as p_pool,
            tc.tile_pool(name="ptq", bufs=2) as ptq_pool,
            tc.tile_pool(name="lp", bufs=2) as lp_pool,
            tc.tile_pool(name="scps", bufs=2, space="PSUM") as sc_psum,
            tc.tile_pool(name="ptps", bufs=2, space="PSUM") as pt_psum,
            tc.tile_pool(name="pops", bufs=2, space="PSUM") as po_psum,
        ):
            ident = singles.tile([P, P], f32)
            make_identity(nc, ident)
            ident_h = singles.tile([P, P], f16)
            nc.vector.tensor_copy(out=ident_h, in_=ident)
            pbias = singles.tile([P, 1], f32)
            nc.vector.memset(pbias, float(np.log(P_SCALE)) if P_SCALE != 1.0 else 0.0)
            wsd_sb = singles.tile([P, DJ, 32], wsdt)
            nc.gpsimd.dma_start(out=wsd_sb, in_=wsd[:, :].rearrange("p (j h) -> p j h", h=32))
            wvT_sb = singles.tile([P, DJ, D], f16)
            owT_sb = singles.tile([P, DJ, D], f16)
            cvec_sb = singles.tile([bloc, D], f32)
            l_acc = singles.tile([P, bloc], f32)
            pooled_all = singles.tile([HP, bloc, D], f32)
            # pooled^T stacked [ki(d), j, b*32+hp]; zero the pad columns once
            pstack = singles.tile([P, DJ, P], f16)
            nc.vector.memset(pstack, 0.0)

            # --- software-pipelined main loop over a tile plan ---
            # Batch 0 ramps up with small tiles so the PE starts after ~0.4 MB
            # of DMA instead of ~3 MB (and the clock gate warms once).
            plan = []  # (b, dram_tile, n0, nlen, bfirst, blast)
            for b in range(bloc):
                if b == 0:
                    sizes = [512, 512, 1024] + [C] * (nt - 1)
                else:
                    sizes = [C] * nt
                off = 0
                for k, nlen in enumerate(sizes):
                    plan.append(
                        (b, b * nt + off // C, off % C, nlen,
                         k == 0, k == len(sizes) - 1)
                    )
                    off += nlen
            NE = len(plan)
            tn_tiles = [None] * NE
            tt_tiles = [None] * NE
            pt_tiles = [None] * NE
            pooled_tiles = {}

            def issue_dma(k):
                b, ud, n0, nlen, bf, bl = plan[k]
                nch = nlen // CH
                mb = nlen // P
                tt = tokt_pool.tile([P, DJ, nlen], f8, tag="tt", name=f"tt{k}")
                src_t = tokt[ud, :, :].rearrange("p (j n) -> p j n", n=C)
                if k == 0:
                    nc.sync.dma_start(
                        out=tt, in_=src_t[:, :, n0 : n0 + nlen]
                    )
                else:
                    nc.sync.dma_start(out=tt, in_=src_t[:, :, n0 : n0 + nlen])
                tn = tokn_pool.tile([P, mb, D], f8, tag="tn", name=f"tn{k}")
                nc.sync.dma_start(
                    out=tn,
                    in_=tokn[ud, :, :].rearrange("p (m d) -> p m d", d=D)[
                        :, n0 // P : n0 // P + mb, :
                    ],
                )
                tn_tiles[k], tt_tiles[k] = tn, tt

            def scores_exp(k):
                b, ud, n0, nlen, bf, bl = plan[k]
                nch = nlen // CH
                tt = tt_tiles[k]
                ps = sc_psum.tile([P, CH], f32, tag="sc", name=f"ps{k}")
                for c in range(nch):
                    for j in range(DJ):
                        nc.tensor.matmul(
                            ps[32 * c : 32 * c + 32, :],
                            wsd_sb[:, j, :],
                            tt[:, j, c * CH : (c + 1) * CH],
                            start=(j == 0),
                            stop=(j == DJ - 1),
                            tile_position=(0, 32 * c),
                        )
                p_t = p_pool.tile([P, CH], f16, tag="p", name=f"p{k}")
                lp = lp_pool.tile([P, 1], f32, tag="l", name=f"lp{k}")
                rows = 32 * nch
                nc.scalar.activation(
                    out=p_t[0:rows, :], in_=ps[0:rows, :], func=EXP,
                    scale=1.0 / WS_SCALE, bias=pbias[0:rows, :],
                    accum_out=lp[0:rows, :],
                )
                nc.vector.tensor_add(
                    out=l_acc[0:rows, b : b + 1],
                    in0=l_acc[0:rows, b : b + 1],
                    in1=lp[0:rows, :],
                )
                pt_tiles[k] = p_t

            def pooled_pass(k):
                b, ud, n0, nlen, bf, bl = plan[k]
                nch = nlen // CH
                mb = nlen // P
                p_t = pt_tiles[k]
                tn = tn_tiles[k]
                if bf:
                    pooled_tiles[b] = po_psum.tile([HP, D], f32, tag="po", name=f"po{b}")
                po = pooled_tiles[b]
                ptq = ptq_pool.tile([P, mb, HP], f16, tag="ptq", name=f"ptq{k}")
                for nblk in range(4):
                    trp = pt_psum.tile([P, P], f16, tag="pt", name=f"trp{k}_{nblk}")
                    nc.tensor.transpose(
                        trp[:, 0 : 32 * nch],
                        p_t[0 : 32 * nch, nblk * P : (nblk + 1) * P],
                        ident_h[0 : 32 * nch, 0 : 32 * nch],
                    )
                    # cols 32c+hp of trp -> ptq slot m = 4c + nblk
                    nc.vector.tensor_copy(
                        out=ptq[:, nblk : nblk + 4 * (nch - 1) + 1 : 4, :],
                        in_=trp[:, 0 : 32 * nch].rearrange(
                            "p (c h) -> p c h", h=32
                        )[:, :, 0:HP],
                    )
                for m in range(mb):
                    st = bf and m == 0
                    sp = bl and m == mb - 1
                    nc.tensor.matmul(
                        po[:, 0:512], ptq[:, m, :], tn[:, m, 0:512],
                        start=st, stop=sp,
                    )
                    nc.tensor.matmul(
                        po[:, 512:768], ptq[:, m, :], tn[:, m, 512:768],
                        start=st, stop=sp,
                    )
                if bl:
                    nc.vector.tensor_copy(out=pooled_all[:, b, :], in_=po)
                    batch_tail(b)

            def batch_tail(b):
                for j in range(DJ):
                    trb = pt_psum.tile([P, HP], f32, tag="pt", name=f"trb{b}_{j}")
                    nc.tensor.transpose(
                        trb,
                        pooled_all[:, b, j * P : (j + 1) * P],
                        ident[:HP, :HP],
                    )
                    nc.vector.tensor_copy(
                        out=pstack[:, j, 32 * b : 32 * b + HP], in_=trb
                    )

            nc.vector.memset(l_acc, 0.0)
            for k0 in range(min(3, NE)):
                issue_dma(k0)
            for k in range(NE):
                if k + 3 < NE:
                    issue_dma(k + 3)
                scores_exp(k)
                if k > 0:
                    pooled_pass(k - 1)
            def emit_l_chain():
                # per-(b, h) softmax denominators -> 1/l as [b*32+hp, 1]
                ltp = pt_psum.tile([bloc, P], f32, tag="pt")
                nc.tensor.transpose(ltp, l_acc, ident)
                lt_sb = singles.tile([bloc, P], f32)
                nc.vector.tensor_copy(out=lt_sb, in_=ltp)
                lsum = singles.tile([bloc, 32], f32)
                nc.vector.tensor_add(
                    out=lsum, in0=lt_sb[:, 0:32], in1=lt_sb[:, 32:64]
                )
                nc.vector.tensor_add(out=lsum, in0=lsum, in1=lt_sb[:, 64:96])
                nc.vector.tensor_add(out=lsum, in0=lsum, in1=lt_sb[:, 96:128])
                linv4 = singles.tile([bloc, 32], f32)
                nc.vector.reciprocal(linv4, lsum)
                lvp = pt_psum.tile([32, bloc], f32, tag="pt")
                nc.tensor.transpose(lvp, linv4, ident[:bloc, :bloc])
                lvp_sb = singles.tile([32, bloc], f32)
                nc.vector.tensor_copy(out=lvp_sb, in_=lvp)
                lbh_ps = pt_psum.tile([P, 1], f32, tag="pt")
                for b in range(bloc):
                    nc.tensor.matmul(
                        lbh_ps[32 * b : 32 * b + 32, :],
                        ident[:32, :32],
                        lvp_sb[:, b : b + 1],
                        start=True, stop=True,
                        tile_position=(0, 32 * b),
                    )
                linv_bh = singles.tile([P, 1], f32)
                nc.vector.memset(linv_bh, 1.0)
                nc.vector.tensor_copy(
                    out=linv_bh[0 : 32 * bloc, :], in_=lbh_ps[0 : 32 * bloc, :]
                )
                return linv_bh

            pooled_pass(NE - 1)
            linv_bh = emit_l_chain()

            nc.gpsimd.dma_start(out=wvT_sb, in_=wvT[:, :].rearrange("p (j d) -> p j d", d=D))
            nc.gpsimd.dma_start(out=owT_sb, in_=owT[:, :].rearrange("p (j d) -> p j d", d=D))
            nc.gpsimd.dma_start(out=cvec_sb, in_=cvec[:, :])

            # ---- tail ----

            # ctx^T[(b,hp), e] = sum_d pstack[d, (b,hp)] wv[e, d], then * 1/l
            ctxT_ps = po_psum.tile([P, D], f32, tag="po")
            for j in range(DJ):
                nc.tensor.matmul(
                    ctxT_ps[:, 0:512], pstack[:, j, :], wvT_sb[:, j, 0:512],
                    start=(j == 0), stop=(j == DJ - 1),
                )
                nc.tensor.matmul(
                    ctxT_ps[:, 512:768], pstack[:, j, :], wvT_sb[:, j, 512:768],
                    start=(j == 0), stop=(j == DJ - 1),
                )
            ctxT_n = singles.tile([P, D], f16)
            nc.vector.tensor_scalar_mul(ctxT_n, ctxT_ps, linv_bh[:, :])

            # transpose ctx^T back to [e, (b,hp)] blocks and select head columns
            ctx_sel = singles.tile([P, DJ, bloc], f16)
            for e in range(DJ):
                c2 = pt_psum.tile([P, P], f16, tag="pt")
                nc.tensor.transpose(
                    c2, ctxT_n[:, e * P : (e + 1) * P], ident_h
                )
                for half in range(2):
                    h = 2 * e + half
                    rows = slice(half * 64, half * 64 + 64)
                    nc.vector.tensor_copy(
                        out=ctx_sel[rows, e, :],
                        in_=c2[rows, :].rearrange("p (b g) -> p b g", g=32)[:, 0:bloc, h],
                    )

            # out[b, o] = sum_e ow[o, e] ctx[e, b] + c  (ctx as 4-col stationary)
            out_ps = po_psum.tile([bloc, D], f32, tag="po")
            for e in range(DJ):
                nc.tensor.matmul(
                    out_ps[:, 0:512], ctx_sel[:, e, :], owT_sb[:, e, 0:512],
                    start=(e == 0), stop=(e == DJ - 1),
                )
                nc.tensor.matmul(
                    out_ps[:, 512:768], ctx_sel[:, e, :], owT_sb[:, e, 512:768],
                    start=(e == 0), stop=(e == DJ - 1),
                )
            fin_sb = singles.tile([bloc, D], f32)
            nc.vector.tensor_add(out=fin_sb, in0=out_ps, in1=cvec_sb)
            nc.sync.dma_start(out=out_d[:, :], in_=fin_sb)
    if legalize:
        _legalize_waits(nc)
    return nc


def host_prep(query, in_proj_w, in_proj_b, out_w, out_b, ws_bf16=False):
    import ml_dtypes

    f8 = ml_dtypes.float8_e3m4
    bf16 = ml_dtypes.bfloat16
    scale = 1.0 / np.sqrt(D // H)
    wq, wk = in_proj_w[:D], in_proj_w[D : 2 * D]
    wv = in_proj_w[2 * D :]
    bq = in_proj_b[:D]
    bv = in_proj_b[2 * D :]
    q_flat = query[0, 0] @ wq.T + bq
    ws = (q_flat.reshape(H, D // H)[:, :, None] * wk.reshape(H, D // H, D)).sum(1)
    ws_scaled = (ws * scale * WS_SCALE).astype(np.float32)  # [H, D]  (WS_SCALE=1)
    wsd = np.zeros((P, DJ, 32), np.float32)
    # wsd[ki, j, hp] = ws_scaled[hp, j*128 + ki]
    wsd[:, :, :H] = ws_scaled.T.reshape(DJ, P, H).transpose(1, 0, 2)
    wsd = np.ascontiguousarray(wsd.reshape(P, DJ * 32)).astype(
        bf16 if ws_bf16 else np.float16
    )
    wvT_r = np.ascontiguousarray(
        wv.T.reshape(DJ, P, D).transpose(1, 0, 2).reshape(P, DJ * D)
    ).astype(np.float16)
    owT_r = np.ascontiguousarray(
        out_w.T.reshape(DJ, P, D).transpose(1, 0, 2).reshape(P, DJ * D)
    ).astype(np.float16)
    cvec_r = np.broadcast_to(
        (out_w @ bv + out_b).astype(np.float32), (BLOC, D)
    ).copy()
    return wsd, wvT_r, owT_r, cvec_r


def shard_tokens(tokens_f8, core, bloc=BLOC, nt=NT):
    """Per-core DMA-optimal token layouts from the full e3m4 token array."""
    tb = tokens_f8[core * bloc : (core + 1) * bloc]  # [bloc, N, D]
    n = nt * C
    tokn = np.ascontiguousarray(
        tb.reshape(bloc, nt, MB, P, D).transpose(0, 1, 3, 2, 4)
    ).reshape(bloc * nt, P, MB * D)
    tokt = np.ascontiguousarray(
        tb.reshape(bloc, nt, C, DJ, P).transpose(0, 1, 4, 3, 2)
    ).reshape(bloc * nt, P, DJ * C)
    return tokn, tokt


def make_in_maps(inputs, ws_bf16=False):
    import ml_dtypes

    tokens = np.asarray(inputs["tokens"], dtype=np.float32)
    wsd, wvT_r, owT_r, cvec_r = host_prep(
        np.asarray(inputs["query"], dtype=np.float32),
        np.asarray(inputs["in_proj_w"], dtype=np.float32),
        np.asarray(inputs["in_proj_b"], dtype=np.float32),
        np.asarray(inputs["out_w"], dtype=np.float32),
        np.asarray(inputs["out_b"], dtype=np.float32),
        ws_bf16=ws_bf16,
    )
    tok8 = tokens.astype(ml_dtypes.float8_e3m4)
    in_maps = []
    for i in range(NCORES):
        tokn, tokt = shard_tokens(tok8, i)
        in_maps.append(
            {
                "tokn": tokn,
                "tokt": tokt,
                "wsd": wsd,
                "wvT": wvT_r,
                "owT": owT_r,
                "cvec": cvec_r,
            }
        )
    return in_maps


def kernel(tokens, query, in_proj_w, in_proj_b, out_w, out_b):
    _patch_tile_drain()
    from concourse.bass_utils import run_bass_kernel_spmd

    inputs = {
        "tokens": tokens, "query": query, "in_proj_w": in_proj_w,
        "in_proj_b": in_proj_b, "out_w": out_w, "out_b": out_b,
    }
    nc = build_nc()
    in_maps = make_in_maps(inputs)
    res = run_bass_kernel_spmd(nc, in_maps, core_ids=list(range(NCORES)))
    return np.concatenate(
        [res.results[i]["out"] for i in range(NCORES)], axis=0
    ).astype(np.float32)


# revision 27
# speedup vs baseline: 1.1840x; 1.1840x over previous
"""AttentionPool Trainium2 kernel (8-core SPMD, batch-sharded, fp8-e3m4).

Math (algebraically folded from the reference):
  The single learned query collapses attention to:
    ws[h,:]   = sum_{d in head h} q_flat[h*64+d] * wk[h*64+d, :]   (host, tiny)
    s[b,h,n]  = tokens[b,n,:] @ (ws[h,:] * scale)                  (device)
    p         = exp(s)            (softmax shift cancels; |s| <~ 3, safe)
    pooled    = (p @ tokens) / sum_n p                             (device)
    ctx[b,e]  = wv[e,:] @ pooled[b, e//64, :];  out = ctx @ out_w.T + c
  Per-head score bias is constant within each softmax row and cancels; all
  other biases fold into c = out_w @ bv + out_b (host).

Device design (per core: 4 batches x 4096 tokens):
  - Tokens streamed twice in fp8 e3m4 (4-bit mantissa): natural layout
    [ki, m, d] for the pooled matmul and host-pre-transposed [ki, j, n] for
    the scores matmul.  25 MB/core total, 1.5 MB per DMA, 12 KB contiguous
    per partition line.
  - Scores for each 2048-token tile land stacked in ONE psum bank:
    chunk c (512 tokens) occupies partitions 32c..32c+16 (12 real heads + 4
    zero-pad).  A single accumulation group spans all 24 matmuls of a tile
    (per-element has_written handles the disjoint partition regions), so exp
    is one [128,512] ACT op and p-transposes are 4x [128,128] per tile.
  - ws is scaled x128 on host before e3m4 (its entries otherwise sit in the
    subnormal range); exp descales via its free `scale` parameter.  p is
    scaled x0.5 via the exp `bias` so its e3m4 cast can't overflow (max 15.5);
    the softmax normalization cancels the factor.
  - Tail: wv/out_w projections in bf16, normalization folded past the wv
    projection via a 0/1 head-selection matmul on 1/l.
"""

import numpy as np

P = 128
D = 768
H = 12
HP = 16          # heads padded to 16 (psum partition block)
DJ = D // P      # 6 chunks of the model dim
C = 2048         # tokens per DMA tile
CH = 512         # tokens per score chunk (one psum bank column span)
NCH = C // CH    # 4 chunks per tile
MB = C // P      # 16 token blocks of 128 per tile
B = 32
N = 4096
NT = N // C      # tiles per batch
NCORES = 8
BLOC = B // NCORES
WS_SCALE = 1.0
P_SCALE = 1.0
WARMUP = 0

_PATCHED = False


def _patch_tile_drain():
    """This walrus build allows only ONE sync wait per instruction (2 for
    EventSemaphore), but TileContext._drain_and_barrier puts a wait per
    outstanding semaphore on the single tail Drain. Split: one Drain each."""
    global _PATCHED
    if _PATCHED:
        return
    import bass_rust
    import concourse.tile as tile
    from concourse.vector_clock import ScopedClock

    def _drain_and_barrier(self, tick_clock, wait_clock):
        nc = self.nc
        probe = nc.sync.drain()
        wait_clock.add_sem_waits(
            probe.ins, ScopedClock({None: tick_clock.global_clock})
        )
        si = probe.ins.sync_info
        if si is not None and len(si.on_wait) > 1:
            waits = list(si.on_wait)
            probe.ins.sync_info = bass_rust.SyncInfo(
                on_wait=[waits[0]], on_update=list(si.on_update)
            )
            for w in waits[1:]:
                extra = nc.sync.drain()
                extra.ins.sync_info = bass_rust.SyncInfo(on_wait=[w], on_update=[])
        nc.all_engine_barrier()
        popped = nc._tile_sem_poison_stack.pop()
        assert popped is self._sem_poison
        nc.clear_and_free_semaphores(list(self.sems.allocated().values()))
        nc.all_engine_barrier()

    tile.TileContext._drain_and_barrier = _drain_and_barrier
    _PATCHED = True


def _legalize_waits(nc):
    """TRN2 walrus encodes at most ONE sync wait per instruction (two for
    EventSemaphore). Tile's wait assignment can leave more; hoist the extras
    onto standalone EventSemaphore instructions inserted just before, on the
    same engine (same semantics: engine blocks on them in order)."""
    import bass_rust
    from concourse import mybir

    n_fixed = 0
    for f in nc.m.functions:
        for bb in f.blocks:
            out = []
            for inst in bb.instructions:
                si = inst.sync_info
                waits = list(si.on_wait) if si is not None else []
                cap = 2 if isinstance(inst, mybir.InstEventSemaphore) else 1
                if len(waits) > cap:
                    extras, keep = waits[:-cap], waits[-cap:]
                    for i in range(0, len(extras), 2):
                        ev = mybir.InstEventSemaphore(
                            name=f"EVW-{inst.name}-{i}", ins=[], outs=[]
                        )
                        ev.engine = inst.engine
                        ev.sync_info = bass_rust.SyncInfo(
                            on_wait=extras[i : i + 2], on_update=[]
                        )
                        out.append(ev)
                    inst.sync_info = bass_rust.SyncInfo(
                        on_wait=keep, on_update=list(si.on_update)
                    )
                    n_fixed += 1
                out.append(inst)
            bb.instructions = out
    return n_fixed


def build_nc(bloc=BLOC, nt=NT, ws_bf16=False, legalize=True):
    import concourse.bass as bass
    import concourse.tile as tile
    from concourse import mybir
    from concourse.masks import make_identity

    f32 = mybir.dt.float32
    f32r = mybir.dt.float32r
    f16 = mybir.dt.float16
    bf16 = mybir.dt.bfloat16
    f8 = mybir.dt.float8e3
    EXP = mybir.ActivationFunctionType.Exp
    NTILES = bloc * nt

    nc = bass.Bass()
    tokn = nc.declare_dram_parameter("tokn", [NTILES, P, MB * D], f8, isOutput=False)
    tokt = nc.declare_dram_parameter("tokt", [NTILES, P, DJ * C], f8, isOutput=False)
    wsdt = bf16 if ws_bf16 else f16
    wsd = nc.declare_dram_parameter("wsd", [P, DJ * 32], wsdt, isOutput=False)
    wvT = nc.declare_dram_parameter("wvT", [P, DJ * D], f16, isOutput=False)
    owT = nc.declare_dram_parameter("owT", [P, DJ * D], f16, isOutput=False)
    cvec = nc.declare_dram_parameter("cvec", [bloc, D], f32, isOutput=False)
    out_d = nc.declare_dram_parameter("out", [bloc, D], f32, isOutput=True)

    with tile.TileContext(nc) as tc:
        with (
            tc.tile_pool(name="singles", bufs=1) as singles,
            tc.tile_pool(name="tokn", bufs=3) as tokn_pool,
            tc.tile_pool(name="tokt", bufs=3) as tokt_pool,
            tc.tile_pool(name="pp", bufs=3) as p_pool,
            tc.tile_pool(name="ptq", bufs=2) as ptq_pool,
            tc.tile_pool(name="lp", bufs=2) as lp_pool,
            tc.tile_pool(name="scps", bufs=2, space="PSUM") as sc_psum,
            tc.tile_pool(name="ptps", bufs=2, space="PSUM") as pt_psum,
            tc.tile_pool(name="pops", bufs=2, space="PSUM") as po_psum,
        ):
            ident = singles.tile([P, P], f32)
            make_identity(nc, ident)
            ident_h = singles.tile([P, P], f16)
            nc.vector.tensor_copy(out=ident_h, in_=ident)
            pbias = singles.tile([P, 1], f32)
            nc.vector.memset(pbias, float(np.log(P_SCALE)) if P_SCALE != 1.0 else 0.0)
            wsd_sb = singles.tile([P, DJ, 32], wsdt)
            nc.gpsimd.dma_start(out=wsd_sb, in_=wsd[:, :].rearrange("p (j h) -> p j h", h=32))
            wvT_sb = singles.tile([P, DJ, D], f16)
            owT_sb = singles.tile([P, DJ, D], f16)
            cvec_sb = singles.tile([bloc, D], f32)
            l_acc = singles.tile([P, bloc], f32)
            pooled_all = singles.tile([HP, bloc, D], f32)
            # pooled^T stacked [ki(d), j, b*32+hp]; zero the pad columns once
            pstack = singles.tile([P, DJ, P], f16)
            nc.vector.memset(pstack, 0.0)

            # --- software-pipelined main loop over a tile plan ---
            # Batch 0 ramps up with small tiles so the PE starts after ~0.4 MB
            # of DMA instead of ~3 MB (and the clock gate warms once).
            plan = []  # (b, dram_tile, n0, nlen, bfirst, blast)
            for b in range(bloc):
                if b == 0:
                    sizes = [512, 512, 1024] + [C] * (nt - 1)
                else:
                    sizes = [C] * nt
                off = 0
                for k, nlen in enumerate(sizes):
                    plan.append(
                        (b, b * nt + off // C, off % C, nlen,
                         k == 0, k == len(sizes) - 1)
                    )
                    off += nlen
            NE = len(plan)
            tn_tiles = [None] * NE
            tt_tiles = [None] * NE
            pt_tiles = [None] * NE
            pooled_tiles = {}

            def issue_dma(k):
                b, ud, n0, nlen, bf, bl = plan[k]
                nch = nlen // CH
                mb = nlen // P
                tt = tokt_pool.tile([P, DJ, nlen], f8, tag="tt", name=f"tt{k}")
                src_t = tokt[ud, :, :].rearrange("p (j n) -> p j n", n=C)
                if k == 0:
                    nc.sync.dma_start(
                        out=tt, in_=src_t[:, :, n0 : n0 + nlen]
                    )
                else:
                    nc.sync.dma_start(out=tt, in_=src_t[:, :, n0 : n0 + nlen])
                tn = tokn_pool.tile([P, mb, D], f8, tag="tn", name=f"tn{k}")
                nc.sync.dma_start(
                    out=tn,
                    in_=tokn[ud, :, :].rearrange("p (m d) -> p m d", d=D)[
                        :, n0 // P : n0 // P + mb, :
                    ],
                )
                tn_tiles[k], tt_tiles[k] = tn, tt

            def scores_exp(k):
                b, ud, n0, nlen, bf, bl = plan[k]
                nch = nlen // CH
                tt = tt_tiles[k]
                ps = sc_psum.tile([P, CH], f32, tag="sc", name=f"ps{k}")
                for c in range(nch):
                    for j in range(DJ):
                        nc.tensor.matmul(
                            ps[32 * c : 32 * c + 32, :],
                            wsd_sb[:, j, :],
                            tt[:, j, c * CH : (c + 1) * CH],
                            start=(j == 0),
                            stop=(j == DJ - 1),
                            tile_position=(0, 32 * c),
                        )
                p_t = p_pool.tile([P, CH], f16, tag="p", name=f"p{k}")
                lp = lp_pool.tile([P, 1], f32, tag="l", name=f"lp{k}")
                rows = 32 * nch
                nc.scalar.activation(
                    out=p_t[0:rows, :], in_=ps[0:rows, :], func=EXP,
                    scale=1.0 / WS_SCALE, bias=pbias[0:rows, :],
                    accum_out=lp[0:rows, :],
                )
                nc.vector.tensor_add(
                    out=l_acc[0:rows, b : b + 1],
                    in0=l_acc[0:rows, b : b + 1],
                    in1=lp[0:rows, :],
                )
                pt_tiles[k] = p_t

            def pooled_pass(k):
                b, ud, n0, nlen, bf, bl = plan[k]
                nch = nlen // CH
                mb = nlen // P
                p_t = pt_tiles[k]
                tn = tn_tiles[k]
                if bf:
                    pooled_tiles[b] = po_psum.tile([HP, D], f32, tag="po", name=f"po{b}")
                po = pooled_tiles[b]
                ptq = ptq_pool.tile([P, mb, HP], f16, tag="ptq", name=f"ptq{k}")
                for nblk in range(4):
                    trp = pt_psum.tile([P, P], f16, tag="pt", name=f"trp{k}_{nblk}")
                    nc.tensor.transpose(
                        trp[:, 0 : 32 * nch],
                        p_t[0 : 32 * nch, nblk * P : (nblk + 1) * P],
                        ident_h[0 : 32 * nch, 0 : 32 * nch],
                    )
                    # cols 32c+hp of trp -> ptq slot m = 4c + nblk
                    nc.vector.tensor_copy(
                        out=ptq[:, nblk : nblk + 4 * (nch - 1) + 1 : 4, :],
                        in_=trp[:, 0 : 32 * nch].rearrange(
                            "p (c h) -> p c h", h=32
                        )[:, :, 0:HP],
                    )
                for m in range(mb):
                    st = bf and m == 0
                    sp = bl and m == mb - 1
                    nc.tensor.matmul(
                        po[:, 0:512], ptq[:, m, :], tn[:, m, 0:512],
                        start=st, stop=sp,
                    )
                    nc.tensor.matmul(
                        po[:, 512:768], ptq[:, m, :], tn[:, m, 512:768],
                        start=st, stop=sp,
                    )
                if bl:
                    nc.vector.tensor_copy(out=pooled_all[:, b, :], in_=po)
                    batch_tail(b)

            def batch_tail(b):
                for j in range(DJ):
                    trb = pt_psum.tile([P, HP], f32, tag="pt", name=f"trb{b}_{j}")
                    nc.tensor.transpose(
                        trb,
                        pooled_all[:, b, j * P : (j + 1) * P],
                        ident[:HP, :HP],
                    )
                    nc.vector.tensor_copy(
                        out=pstack[:, j, 32 * b : 32 * b + HP], in_=trb
                    )

            nc.vector.memset(l_acc, 0.0)
            issue_dma(0)
            if NE > 1:
                issue_dma(1)
            for k in range(NE):
                if k + 2 < NE:
                    issue_dma(k + 2)
                scores_exp(k)
                if k > 0:
                    pooled_pass(k - 1)
            def emit_l_chain():
                # per-(b, h) softmax denominators -> 1/l as [b*32+hp, 1]
                ltp = pt_psum.tile([bloc, P], f32, tag="pt")
                nc.tensor.transpose(ltp, l_acc, ident)
                lt_sb = singles.tile([bloc, P], f32)
                nc.vector.tensor_copy(out=lt_sb, in_=ltp)
                lsum = singles.tile([bloc, 32], f32)
                nc.vector.tensor_add(
                    out=lsum, in0=lt_sb[:, 0:32], in1=lt_sb[:, 32:64]
                )
                nc.vector.tensor_add(out=lsum, in0=lsum, in1=lt_sb[:, 64:96])
                nc.vector.tensor_add(out=lsum, in0=lsum, in1=lt_sb[:, 96:128])
                linv4 = singles.tile([bloc, 32], f32)
                nc.vector.reciprocal(linv4, lsum)
                lvp = pt_psum.tile([32, bloc], f32, tag="pt")
                nc.tensor.transpose(lvp, linv4, ident[:bloc, :bloc])
                lvp_sb = singles.tile([32, bloc], f32)
                nc.vector.tensor_copy(out=lvp_sb, in_=lvp)
                lbh_ps = pt_psum.tile([P, 1], f32, tag="pt")
                for b in range(bloc):
                    nc.tensor.matmul(
                        lbh_ps[32 * b : 32 * b + 32, :],
                        ident[:32, :32],
                        lvp_sb[:, b : b + 1],
                        start=True, stop=True,
                        tile_position=(0, 32 * b),
                    )
                linv_bh = singles.tile([P, 1], f32)
                nc.vector.memset(linv_bh, 1.0)
                nc.vector.tensor_copy(
                    out=linv_bh[0 : 32 * bloc, :], in_=lbh_ps[0 : 32 * bloc, :]
                )
                return linv_bh

            pooled_pass(NE - 1)
            linv_bh = emit_l_chain()

            nc.gpsimd.dma_start(out=wvT_sb, in_=wvT[:, :].rearrange("p (j d) -> p j d", d=D))
            nc.gpsimd.dma_start(out=owT_sb, in_=owT[:, :].rearrange("p (j d) -> p j d", d=D))
            nc.gpsimd.dma_start(out=cvec_sb, in_=cvec[:, :])

            # ---- tail ----

            # ctx^T[(b,hp), e] = sum_d pstack[d, (b,hp)] wv[e, d], then * 1/l
            ctxT_ps = po_psum.tile([P, D], f32, tag="po")
            for j in range(DJ):
                nc.tensor.matmul(
                    ctxT_ps[:, 0:512], pstack[:, j, :], wvT_sb[:, j, 0:512],
                    start=(j == 0), stop=(j == DJ - 1),
                )
                nc.tensor.matmul(
                    ctxT_ps[:, 512:768], pstack[:, j, :], wvT_sb[:, j, 512:768],
                    start=(j == 0), stop=(j == DJ - 1),
                )
            ctxT_n = singles.tile([P, D], f16)
            nc.vector.tensor_scalar_mul(ctxT_n, ctxT_ps, linv_bh[:, :])

            # transpose ctx^T back to [e, (b,hp)] blocks and select head columns
            ctx_sel = singles.tile([P, DJ, bloc], f16)
            for e in range(DJ):
                c2 = pt_psum.tile([P, P], f16, tag="pt")
                nc.tensor.transpose(
                    c2, ctxT_n[:, e * P : (e + 1) * P], ident_h
                )
                for half in range(2):
                    h = 2 * e + half
                    rows = slice(half * 64, half * 64 + 64)
                    nc.vector.tensor_copy(
                        out=ctx_sel[rows, e, :],
                        in_=c2[rows, :].rearrange("p (b g) -> p b g", g=32)[:, 0:bloc, h],
                    )

            # out[b, o] = sum_e ow[o, e] ctx[e, b] + c  (ctx as 4-col stationary)
            out_ps = po_psum.tile([bloc, D], f32, tag="po")
            for e in range(DJ):
                nc.tensor.matmul(
                    out_ps[:, 0:512], ctx_sel[:, e, :], owT_sb[:, e, 0:512],
                    start=(e == 0), stop=(e == DJ - 1),
                )
                nc.tensor.matmul(
                    out_ps[:, 512:768], ctx_sel[:, e, :], owT_sb[:, e, 512:768],
                    start=(e == 0), stop=(e == DJ - 1),
                )
            fin_sb = singles.tile([bloc, D], f32)
            nc.vector.tensor_add(out=fin_sb, in0=out_ps, in1=cvec_sb)
            nc.sync.dma_start(out=out_d[:, :], in_=fin_sb)
    if legalize:
        _legalize_waits(nc)
    return nc


def host_prep(query, in_proj_w, in_proj_b, out_w, out_b, ws_bf16=False):
    import ml_dtypes

    f8 = ml_dtypes.float8_e3m4
    bf16 = ml_dtypes.bfloat16
    scale = 1.0 / np.sqrt(D // H)
    wq, wk = in_proj_w[:D], in_proj_w[D : 2 * D]
    wv = in_proj_w[2 * D :]
    bq = in_proj_b[:D]
    bv = in_proj_b[2 * D :]
    q_flat = query[0, 0] @ wq.T + bq
    ws = (q_flat.reshape(H, D // H)[:, :, None] * wk.reshape(H, D // H, D)).sum(1)
    ws_scaled = (ws * scale * WS_SCALE).astype(np.float32)  # [H, D]  (WS_SCALE=1)
    wsd = np.zeros((P, DJ, 32), np.float32)
    # wsd[ki, j, hp] = ws_scaled[hp, j*128 + ki]
    wsd[:, :, :H] = ws_scaled.T.reshape(DJ, P, H).transpose(1, 0, 2)
    wsd = np.ascontiguousarray(wsd.reshape(P, DJ * 32)).astype(
        bf16 if ws_bf16 else np.float16
    )
    wvT_r = np.ascontiguousarray(
        wv.T.reshape(DJ, P, D).transpose(1, 0, 2).reshape(P, DJ * D)
    ).astype(np.float16)
    owT_r = np.ascontiguousarray(
        out_w.T.reshape(DJ, P, D).transpose(1, 0, 2).reshape(P, DJ * D)
    ).astype(np.float16)
    cvec_r = np.broadcast_to(
        (out_w @ bv + out_b).astype(np.float32), (BLOC, D)
    ).copy()
    return wsd, wvT_r, owT_r, cvec_r


def shard_tokens(tokens_f8, core, bloc=BLOC, nt=NT):
    """Per-core DMA-optimal token layouts from the full e3m4 token array."""
    tb = tokens_f8[core * bloc : (core + 1) * bloc]  # [bloc, N, D]
    n = nt * C
    tokn = np.ascontiguousarray(
        tb.reshape(bloc, nt, MB, P, D).transpose(0, 1, 3, 2, 4)
    ).reshape(bloc * nt, P, MB * D)
    tokt = np.ascontiguousarray(
        tb.reshape(bloc, nt, C, DJ, P).transpose(0, 1, 4, 3, 2)
    ).reshape(bloc * nt, P, DJ * C)
    return tokn, tokt


def make_in_maps(inputs, ws_bf16=False):
    import ml_dtypes

    tokens = np.asarray(inputs["tokens"], dtype=np.float32)
    wsd, wvT_r, owT_r, cvec_r = host_prep(
        np.asarray(inputs["query"], dtype=np.float32),
        np.asarray(inputs["in_proj_w"], dtype=np.float32),
        np.asarray(inputs["in_proj_b"], dtype=np.float32),
        np.asarray(inputs["out_w"], dtype=np.float32),
        np.asarray(inputs["out_b"], dtype=np.float32),
        ws_bf16=ws_bf16,
    )
    tok8 = tokens.astype(ml_dtypes.float8_e3m4)
    in_maps = []
    for i in range(NCORES):
        tokn, tokt = shard_tokens(tok8, i)
        in_maps.append(
            {
                "tokn": tokn,
                "tokt": tokt,
                "wsd": wsd,
                "wvT": wvT_r,
                "owT": owT_r,
                "cvec": cvec_r,
            }
        )
    return in_maps


def kernel(tokens, query, in_proj_w, in_proj_b, out_w, out_b):
    _patch_tile_drain()
    from concourse.bass_utils import run_bass_kernel_spmd

    inputs = {
        "tokens": tokens, "query": query, "in_proj_w": in_proj_w,
        "in_proj_b": in_proj_b, "out_w": out_w, "out_b": out_b,
    }
    nc = build_nc()
    in_maps = make_in_maps(inputs)
    res = run_bass_kernel_spmd(nc, in_maps, core_ids=list(range(NCORES)))
    return np.concatenate(
        [res.results[i]["out"] for i in range(NCORES)], axis=0
    ).astype(np.float32)
